# revision 4
# baseline (speedup 1.0000x reference)
"""EnhancedMultiHeadAttention on 8 Trainium2 NeuronCores, v2.

Sharding: 8 cores = 2 batches x 4 head-groups (4 heads / 256 columns each).
All matmul operands are bf16 (fp32 PSUM accumulation).  temporal_bias is
folded into the softmax exp as a per-partition (per-key) activation bias, so
V' needs only a constant ones column for the denominator.  V is projected
directly into [token, head*depth] layout (no PE transposes).  Projections,
attention, normalization and the output projection are interleaved in one
stream so the PE never drains: input DMAs are issued upfront in priority
order (wk/wq, xk, xq[qb0], wv, xv, wo, xq[rest]) and attention on query
block 0 starts as soon as kT and qT[qb0] exist.
"""

import sys

for _p in ("/opt/trn_rl_repo", "/root/.axon_site/_ro/trn_rl_repo"):
    if _p not in sys.path:
        sys.path.append(_p)

import ml_dtypes
import numpy as np

import concourse.bass as bass
import concourse.mybir as mybir
import concourse.tile as tile
from concourse import bacc
from concourse.bass_utils import run_bass_kernel_spmd

F32 = mybir.dt.float32
BF16 = mybir.dt.bfloat16
EXP = mybir.ActivationFunctionType.Exp
IDENT = mybir.ActivationFunctionType.Identity

B, S, D = 2, 2048, 1024
H, DEPTH = 16, 64
NCORES = 8
GROUPS = 4                  # head-groups per batch
HC = H // GROUPS            # heads per core = 4
C = HC * DEPTH              # columns per core = 256
NPAIR = HC // 2             # head pairs per core = 2
DT = D // 128               # 8 d-tiles
TB = S // 512               # 4 token blocks (512 wide)
QB = S // 512               # 4 q blocks
KT = S // 128               # 16 k tiles
SCALE = 0.125               # 1/sqrt(DEPTH)
SKEW = 2                    # positions between scores and AV of same tile
NORM_MODE = "fast"


def build_nc():
    nc = bacc.Bacc(None, target_bir_lowering=False)

    xq = nc.dram_tensor("xq", [D, S], BF16, kind="ExternalInput")
    xk = nc.dram_tensor("xk", [D, S], BF16, kind="ExternalInput")
    xv = nc.dram_tensor("xv", [D, S], BF16, kind="ExternalInput")
    wq = nc.dram_tensor("wq", [128, DT, C], BF16, kind="ExternalInput")
    wk = nc.dram_tensor("wk", [128, DT, C], BF16, kind="ExternalInput")
    wv = nc.dram_tensor("wv", [128, DT, C], BF16, kind="ExternalInput")
    wo = nc.dram_tensor("wo", [128, 2, D], BF16, kind="ExternalInput")
    bq = nc.dram_tensor("bq", [C], F32, kind="ExternalInput")
    bk = nc.dram_tensor("bk", [C], F32, kind="ExternalInput")
    bv = nc.dram_tensor("bv", [1, C], BF16, kind="ExternalInput")
    tbias = nc.dram_tensor("tbias", [S], F32, kind="ExternalInput")
    out = nc.dram_tensor("out", [S, D], F32, kind="ExternalOutput")

    with tile.TileContext(nc) as tc, nc.allow_low_precision(
        reason="bf16 operands with fp32 PSUM accumulation; tolerance 2e-2"
    ):
        with (
            tc.tile_pool(name="wpool", bufs=1) as wp,
            tc.tile_pool(name="xpool", bufs=1) as xp,
            tc.tile_pool(name="qk", bufs=1) as qkp,
            tc.tile_pool(name="vsb", bufs=1) as vp,
            tc.tile_pool(name="ctxp", bufs=1) as cxp,
            tc.tile_pool(name="pex", bufs=8) as pex,
            tc.tile_pool(name="nrm", bufs=4) as nrm,
            tc.tile_pool(name="sps", bufs=2, space="PSUM") as sps,
            tc.tile_pool(name="avp", bufs=1, space="PSUM") as avp,
            tc.tile_pool(name="wps", bufs=2, space="PSUM") as wps,
            tc.tile_pool(name="dsc", bufs=4, space="DRAM") as dsc,
        ):
            # ---- small constants ----
            bq_sb = wp.tile([128, 2], F32)
            bk_sb = wp.tile([128, 2], F32)
            tb_sb = wp.tile([128, KT], F32)
            bv_row = wp.tile([1, C], BF16)
            ones_sb = wp.tile([1, 128], BF16)
            zeros_sb = wp.tile([128, 65], BF16)
            nc.sync.dma_start(bq_sb[:], bq.rearrange("(ct p) -> p ct", p=128))
            nc.sync.dma_start(bk_sb[:], bk.rearrange("(ct p) -> p ct", p=128))
            nc.sync.dma_start(tb_sb[:], tbias.rearrange("(tt p) -> p tt", p=128))
            nc.sync.dma_start(bv_row[:], bv[:, :])
            nc.vector.memset(ones_sb[:], 1.0)
            nc.vector.memset(zeros_sb[:], 0.0)

            # ---- weights ----
            wq_sb = wp.tile([128, DT, C], BF16)
            wk_sb = wp.tile([128, DT, C], BF16)
            wv_sb = wp.tile([128, DT, C], BF16)
            wo_sb = wp.tile([128, 2, D], BF16)
            nc.scalar.dma_start(wk_sb[:], wk[:, :, :])
            nc.scalar.dma_start(wq_sb[:], wq[:, :, :])
            # (queue order continues below: xq0, wv, xv tb0/1, wo, xq tb1-3)

            # ---- all x DMAs upfront on three parallel queues ----
            # sync (SP HWDGE): xk;  scalar (ACT HWDGE): wv + xv;
            # gpsimd (SWDGE): xq + wo.
            xk_r = xk.rearrange("(dt p) t -> dt p t", p=128)
            xq_r = xq.rearrange("(dt p) t -> dt p t", p=128)
            xv_r = xv.rearrange("(dt p) t -> dt p t", p=128)
            xkt = {}
            xqt = {}
            xvt = {}
            for tb in range(TB):
                tsl = slice(tb * 512, (tb + 1) * 512)
                for dt in range(DT):
                    xkt[tb, dt] = xp.tile([128, 512], BF16, name=f"xk{tb}_{dt}")
                    nc.sync.dma_start(xkt[tb, dt][:], xk_r[dt, :, tsl])
            for dt in range(DT):
                xqt[0, dt] = xp.tile([128, 512], BF16, name=f"xq0_{dt}")
                nc.scalar.dma_start(xqt[0, dt][:], xq_r[dt, :, 0:512])
            nc.scalar.dma_start(wv_sb[:], wv[:, :, :])
            # xv rides the gpsimd SWDGE ring: separate from the 8 shared
            # HWDGE queues, and the gpsimd engine is otherwise idle early
            for tb in range(TB):
                tsl = slice(tb * 512, (tb + 1) * 512)
                for dt in range(DT):
                    xvt[tb, dt] = xp.tile([128, 512], BF16, name=f"xv{tb}_{dt}")
                    nc.gpsimd.dma_start(xvt[tb, dt][:], xv_r[dt, :, tsl])
            for tb in range(1, TB):
                tsl = slice(tb * 512, (tb + 1) * 512)
                for dt in range(DT):
                    xqt[tb, dt] = xp.tile([128, 512], BF16, name=f"xq{tb}_{dt}")
                    nc.sync.dma_start(xqt[tb, dt][:], xq_r[dt, :, tsl])
            nc.sync.dma_start(wo_sb[:], wo[:, :, :])

            # ---- persistent activations ----
            qT = [qkp.tile([128, S], BF16, name=f"qT{i}") for i in range(NPAIR)]
            kT = [qkp.tile([128, S], BF16, name=f"kT{i}") for i in range(NPAIR)]
            ctx = [cxp.tile([128, S], BF16, name=f"ctx{i}") for i in range(NPAIR)]
            # V'': [tok, head, 64 V cols + ones col]
            vs = [vp.tile([128, HC, 65], BF16, name=f"vs{t}") for t in range(KT)]

            # ============ projection emitters ============
            def proj_qk(xt, w_sb, b_sb, dst, tb, act_copy):
                tsl = slice(tb * 512, (tb + 1) * 512)
                ps = {}
                for dt in range(DT):
                    for ct in range(2):
                        if dt == 0:
                            ps[ct] = wps.tile([128, 512], F32, tag="pp", name="ps")
                        nc.tensor.matmul(
                            ps[ct][:],
                            w_sb[:, dt, ct * 128 : (ct + 1) * 128],
                            xt[tb, dt][:],
                            start=(dt == 0),
                            stop=(dt == DT - 1),
                        )
                for ct in range(2):
                    if act_copy:
                        nc.scalar.activation(
                            dst[ct][:, tsl], ps[ct][:], IDENT,
                            bias=b_sb[:, ct : ct + 1], scale=1.0,
                        )
                    else:
                        nc.vector.tensor_scalar_add(
                            dst[ct][:, tsl], ps[ct][:], b_sb[:, ct : ct + 1]
                        )

            def proj_qk_half(xt, w_sb, b_sb, dst, tb, ct):
                tsl = slice(tb * 512, (tb + 1) * 512)
                ps = wps.tile([128, 512], F32, tag="pp", name="ps")
                for dt in range(DT):
                    nc.tensor.matmul(
                        ps[:],
                        w_sb[:, dt, ct * 128 : (ct + 1) * 128],
                        xt[tb, dt][:],
                        start=(dt == 0),
                        stop=(dt == DT - 1),
                    )
                nc.vector.tensor_scalar_add(
                    dst[ct][:, tsl], ps[:], b_sb[:, ct : ct + 1]
                )

            def proj_v_block(kt):
                # one 128-token block: out[tok, c] accumulated over d
                tb, sub = divmod(kt, 4)
                ssl = slice(sub * 128, (sub + 1) * 128)
                ps = wps.tile([128, 512], F32, tag="pp", name="vps")
                pv = ps[:, 0:C]
                nc.tensor.matmul(
                    pv, ones_sb[0:1, :], bv_row[0:1, :], start=True, stop=False
                )
                for dt in range(DT):
                    nc.tensor.matmul(
                        pv,
                        xvt[tb, dt][:, ssl],
                        wv_sb[:, dt, :],
                        start=False,
                        stop=(dt == DT - 1),
                    )
                nc.vector.tensor_copy(
                    vs[kt][:, :, 0:64], pv.rearrange("p (h d) -> p h d", h=HC)
                )
                nc.vector.memset(vs[kt][:, :, 64:65], 1.0)

            # ============ attention emitters ============
            pending = {}
            avs = {}

            def emit_qk(qb, pr, kt):
                qsl = slice(qb * 512, (qb + 1) * 512)
                ksl = slice(kt * 128, (kt + 1) * 128)
                st = sps.tile([128, 1024], F32, tag="s", name="st")
                for hh in range(2):
                    psl = slice(hh * 64, (hh + 1) * 64)
                    nc.tensor.matmul(
                        st[:, hh * 512 : (hh + 1) * 512],
                        kT[pr][psl, ksl],
                        qT[pr][psl, qsl],
                    )
                pe = pex.tile([128, 1024], BF16, tag="pe", name="pe")
                nc.scalar.activation(
                    pe[:], st[:], EXP, bias=tb_sb[:, kt : kt + 1], scale=SCALE
                )
                pending[qb, pr, kt] = pe

            def emit_av(qb, pr, kt, pos=None):  # pos unused
                pe = pending.pop((qb, pr, kt))
                if kt == 0:
                    avs[qb, pr] = [
                        avp.tile([65, 512], F32, tag=f"av{hh}", name=f"av{hh}")
                        for hh in range(2)
                    ]
                av = avs[qb, pr]
                for hh in range(2):
                    nc.tensor.matmul(
                        av[hh][:],
                        vs[kt][:, pr * 2 + hh, :],
                        pe[:, hh * 512 : (hh + 1) * 512],
                        start=(kt == 0),
                        stop=(kt == KT - 1),
                    )
                if (qb, pr) != (0, 0) and kt < KT - 1:
                    # HAM ballast: av += 0 keeps the PE streaming (an idle
                    # PE re-throttles to 1.2 GHz, costing far more than the
                    # extra 0.2us/position this no-op burns)
                    nc.tensor.matmul(
                        av[0][:],
                        zeros_sb[:],
                        pe[:, 0:512],
                        start=False,
                        stop=False,
                    )
                if kt == KT - 1:
                    emit_norm(qb, pr)

            def emit_norm(qb, pr):
                if NORM_MODE == "fast":
                    return emit_norm_fast(qb, pr)
                qsl = slice(qb * 512, (qb + 1) * 512)
                av = avs.pop((qb, pr))
                for hh in range(2):
                    # stage av out of PSUM quickly to free the bank
                    a_sb = nrm.tile([65, 512], F32, tag="asb", name="asb")
                    nc.vector.tensor_copy(a_sb[:], av[hh][:])
                    rcb = nrm.tile([64, 512], F32, tag="rcb", name="rcb")
                    rr = nrm.tile([65, 512], F32, tag="rr", name="rr")
                    nc.vector.reciprocal(rr[:], a_sb[:])
                    dr = dsc.tile([1, 512], F32, tag="dr", name="dr")
                    nc.sync.dma_start(dr[:], rr[64:65, :])
                    dr_ap = dr[:]
                    bcast = bass.AP(
                        tensor=dr_ap.tensor, offset=dr_ap.offset,
                        ap=[[0, 64]] + [list(a) for a in dr_ap.ap[1:]],
                    )
                    nc.sync.dma_start(rcb[:], bcast)
                    if hh == 0:
                        nc.vector.tensor_mul(
                            ctx[pr][0:64, qsl], a_sb[0:64, :], rcb[:]
                        )
                    else:
                        tmp = nrm.tile([64, 512], BF16, tag="tmp", name="tmp")
                        nc.vector.tensor_mul(tmp[:], a_sb[0:64, :], rcb[:])
                        nc.sync.dma_start(ctx[pr][64:128, qsl], tmp[:])

            def emit_norm_fast(qb, pr):
                qsl = slice(qb * 512, (qb + 1) * 512)
                av = avs.pop((qb, pr))
                # stage both heads out of PSUM quickly to free the banks
                a_sb = [nrm.tile([65, 512], F32, tag="asb", name="asb", bufs=4)
                        for _ in range(2)]
                # gather the two denominator rows at partitions 0/32 (engine
                # partition bases must be 32-aligned) via small SBUF DMAs —
                # DMA moves partitions; gpsimd software copies are too slow.
                # dma0 runs while the second staging copy is still on DVE.
                den2 = nrm.tile([33, 512], F32, tag="den2", name="den2", bufs=2)
                nc.vector.memset(den2[:], 1.0)
                nc.vector.tensor_copy(a_sb[0][:], av[0][:])
                nc.sync.dma_start(den2[0:1, :], a_sb[0][64:65, :])
                nc.vector.tensor_copy(a_sb[1][:], av[1][:])
                nc.sync.dma_start(den2[32:33, :], a_sb[1][64:65, :])
                rc2 = nrm.tile([33, 512], F32, tag="rc2", name="rc2", bufs=2)
                nc.vector.reciprocal(rc2[:], den2[:])
                # hh0 completes first; hh1 needs a partition hop for broadcast
                rcb0 = nrm.tile([64, 512], F32, tag="rcb", name="rcb", bufs=2)
                nc.gpsimd.partition_broadcast(rcb0[:], rc2[0:1, :])
                nc.vector.tensor_mul(ctx[pr][0:64, qsl], a_sb[0][0:64, :], rcb0[:])
                r1 = nrm.tile([1, 512], F32, tag="r1", name="r1", bufs=2)
                nc.sync.dma_start(r1[:], rc2[32:33, :])
                rcb1 = nrm.tile([64, 512], F32, tag="rcb", name="rcb", bufs=2)
                nc.gpsimd.partition_broadcast(rcb1[:], r1[:])
                tmp = nrm.tile([64, 512], BF16, tag="tmp", name="tmp")
                nc.vector.tensor_mul(tmp[:], a_sb[1][0:64, :], rcb1[:])
                nc.sync.dma_start(ctx[pr][64:128, qsl], tmp[:])

            oout = out.rearrange("(qt p) n -> qt p n", p=128)

            def emit_outproj(qb, j, late=False):
                qt = qb * 4 + j // 2
                n = j % 2
                qts = slice(qt * 128, (qt + 1) * 128)
                po = wps.tile([128, 512], F32, tag="pp", name="po")
                for ct in range(2):
                    nc.tensor.matmul(
                        po[:],
                        ctx[ct][:, qts],
                        wo_sb[:, ct, n * 512 : (n + 1) * 512],
                        start=(ct == 0),
                        stop=(ct == 1),
                    )
                ot = nrm.tile([128, 512], F32, tag="ot", name="ot", bufs=3)
                if late:
                    # tail: ACT and the scalar DMA queue are idle after exps
                    nc.scalar.activation(ot[:], po[:], IDENT)
                    nc.scalar.dma_start(oout[qt][:, n * 512 : (n + 1) * 512], ot[:])
                else:
                    nc.vector.tensor_copy(ot[:], po[:])
                    nc.sync.dma_start(oout[qt][:, n * 512 : (n + 1) * 512], ot[:])

            # ============ phase 1: K[tb0] + Q[qb0] ============
            proj_qk(xkt, wk_sb, bk_sb, kT, 0, act_copy=True)
            proj_qk(xqt, wq_sb, bq_sb, qT, 0, act_copy=False)

            # ============ phase 2: attention stream ============
            flat = [
                (qb, pr, kt)
                for qb in range(QB)
                for pr in range(NPAIR)
                for kt in range(KT)
            ]
            extras = {}

            def add_extra(pos, fn):
                extras.setdefault(pos, []).append(fn)

            for tb in range(1, TB):
                for ct in range(2):
                    add_extra((tb - 1) * 2 + ct,
                              lambda tb=tb, ct=ct: proj_qk_half(
                                  xkt, wk_sb, bk_sb, kT, tb, ct))
            for kt in range(KT):
                add_extra(max(kt, 2), lambda kt=kt: proj_v_block(kt))
            add_extra(22, lambda: proj_qk(xqt, wq_sb, bq_sb, qT, 1, act_copy=False))
            add_extra(54, lambda: proj_qk(xqt, wq_sb, bq_sb, qT, 2, act_copy=False))
            add_extra(86, lambda: proj_qk(xqt, wq_sb, bq_sb, qT, 3, act_copy=False))
            for qb in range(QB - 1):
                base = (qb + 1) * 2 * KT
                for j, off in enumerate((20, 21, 23, 24, 26, 27, 29, 30)):
                    add_extra(base + off, lambda qb=qb, j=j: emit_outproj(qb, j))

            # positions already PE-filled by chain/qproj extras: adding
            # ballast there would overshoot the ACT pace and slow the stream
            ballast_skip = set()
            for qb in range(QB - 1):
                for off in (20, 21, 23, 24, 26, 27, 29, 30):
                    ballast_skip.add((qb + 1) * 2 * KT + off)
            ballast_skip.update((22, 54, 86))

            # AV emission: delay each unit's first AVs so the previous unit's
            # PSUM staging copies have retired before the new accumulation
            # group asks for the banks.
            av_at = {}
            for i, (qb, pr, kt) in enumerate(flat):
                unit_start = (i // KT) * KT
                av_at.setdefault(unit_start + max(kt + SKEW, 7), []).append(i)

            for i, (qb, pr, kt) in enumerate(flat):
                emit_qk(qb, pr, kt)
                for fn in extras.get(i, ()):
                    fn()
                for j in av_at.get(i, ()):
                    emit_av(*flat[j], pos=i)
            pending_ballast = None
            for p in sorted(k for k in av_at if k >= len(flat)):
                for j in av_at[p]:
                    pending_ballast = pending.get(flat[j]) or pending_ballast
                    emit_av(*flat[j])
            # keep the PE streaming through the final norm wait (an idle
            # PE re-throttles to 1.2 GHz and the tail chains would run cold);
            # dead writes into the retired scores PSUM, never read
            for _ in range(52):
                stb = sps.tile([128, 1024], F32, tag="s", name="stb")
                nc.tensor.matmul(
                    stb[0:65, 0:512], zeros_sb[:], pending_ballast[:, 0:512],
                    start=True, stop=True,
                )
            for j in range(8):
                emit_outproj(QB - 1, j, late=True)

    nc.finalize()
    return nc


_NC = None


def _get_nc():
    global _NC
    if _NC is None:
        _NC = build_nc()
    return _NC


def _packw(w, bf):
    # [D, C] -> [128, DT, C]
    return np.ascontiguousarray(
        w.reshape(DT, 128, C).transpose(1, 0, 2).astype(bf))


def _packo(w, bf):
    # [C, D] -> [128, 2, D]
    return np.ascontiguousarray(
        w.reshape(2, 128, D).transpose(1, 0, 2).astype(bf))


def make_in_maps(query, key, value, temporal_bias, wq, bq, wk, bk, wv, bv, wo, bo):
    f = np.float32
    bf = ml_dtypes.bfloat16
    xt = {}
    for b in range(B):
        xt["q", b] = np.ascontiguousarray(np.asarray(query[b], f).T.astype(bf))
        xt["k", b] = np.ascontiguousarray(np.asarray(key[b], f).T.astype(bf))
        xt["v", b] = np.ascontiguousarray(np.asarray(value[b], f).T.astype(bf))
    tb = np.asarray(temporal_bias, f)
    in_maps = []
    for core in range(NCORES):
        b, g = divmod(core, GROUPS)
        cs = slice(g * C, (g + 1) * C)
        in_maps.append({
            "xq": xt["q", b],
            "xk": xt["k", b],
            "xv": xt["v", b],
            "wq": _packw(np.asarray(wq, f)[:, cs], bf),
            "wk": _packw(np.asarray(wk, f)[:, cs], bf),
            "wv": _packw(np.asarray(wv, f)[:, cs], bf),
            "wo": _packo(np.asarray(wo, f)[cs, :], bf),
            "bq": np.ascontiguousarray(np.asarray(bq, f)[cs]),
            "bk": np.ascontiguousarray(np.asarray(bk, f)[cs]),
            "bv": np.ascontiguousarray(np.asarray(bv, f)[cs].astype(bf))[None, :],
            "tbias": np.ascontiguousarray(tb[b]),
        })
    return in_maps


def gather(results, bo):
    bo = np.asarray(bo, np.float32)
    out = np.zeros((B, S, D), np.float32)
    for core in range(NCORES):
        b = core // GROUPS
        out[b] += results[core]["out"]
    out += bo[None, None, :]
    return out


def kernel(query, key, value, temporal_bias, wq, bq, wk, bk, wv, bv, wo, bo,
           _trace=False):
    nc = _get_nc()
    in_maps = make_in_maps(query, key, value, temporal_bias,
                           wq, bq, wk, bk, wv, bv, wo, bo)
    res = run_bass_kernel_spmd(nc, in_maps, list(range(NCORES)), trace=_trace)
    out = gather(res.results, bo)
    if _trace:
        return out, res
    return out
